# revision 1
# baseline (speedup 1.0000x reference)
"""Bass/Tile Trainium2 kernel for batched self-attention:

    O[b] = softmax(tail[b] @ head[b].T / sqrt(D)) @ tail[b]

with B=8, S=2048, D=1024, fp32 in/out.

Strategy
--------
Data-parallel over batch: one batch per NeuronCore (8 cores).

Per core, all matmuls run on TensorE in bf16 with fp32 PSUM
accumulation. The softmax is computed WITHOUT max-subtraction: scores
after the 1/32 temperature are ~N(0,1) (observed |max| < 7 for this
problem's randn inputs), so exp() cannot overflow fp32 and softmax is
shift-invariant anyway.

The kernel computes S^T = (head @ tail^T)/32 tiles with the key axis h
on PSUM partitions and the query axis t on the free axis, applies exp
on ScalarE (PSUM->SBUF, bf16 out), and accumulates

    O^T[d, t] = sum_h tail[h, d] * E[h, t]        (TensorE, PSUM accum)
    den[t]    = sum_h E[h, t]                     (ones-vector matmul)

so NO transposes are needed on device: the host passes head^T / tail^T
(plus tail in natural layout) per core and transposes O^T back on
gather. Normalization (multiply by 1/den broadcast across partitions)
runs on VectorE.
"""

import os
import sys
import contextlib
import ctypes
import types

sys.path.insert(0, "/opt/trn_rl_repo")

import numpy as np
import ml_dtypes


# ---------------------------------------------------------------------------
# NTFF profiling shim: recreate the missing antenv.axon_hooks module so
# run_bass_kernel_spmd(trace=True) can capture HW profiles under axon.
# Only used when BASS_ATTN_TRACE=1; harmless otherwise.
# ---------------------------------------------------------------------------
def _install_ntff_shim():
    if "antenv.axon_hooks" in sys.modules:
        return
    so_path = "/opt/axon/libaxon_pjrt.so"
    hook = None
    try:
        lib = ctypes.CDLL(so_path)
        if hasattr(lib, "axon_start_nrt_profile"):
            lib.axon_start_nrt_profile.argtypes = [
                ctypes.POINTER(ctypes.c_int64),
                ctypes.c_size_t,
            ]
            lib.axon_start_nrt_profile.restype = ctypes.c_int64
            lib.axon_stop_nrt_profile.argtypes = [ctypes.c_char_p]
            lib.axon_stop_nrt_profile.restype = ctypes.c_int64

            @contextlib.contextmanager
            def _hook(output_dir, device_ids):
                import jax

                jax.devices()
                if device_ids:
                    ids = (ctypes.c_int64 * len(device_ids))(*device_ids)
                    rc = lib.axon_start_nrt_profile(ids, len(device_ids))
                else:
                    rc = lib.axon_start_nrt_profile(None, 0)
                if rc != 0:
                    raise RuntimeError(f"axon_start_nrt_profile rc={rc}")
                try:
                    yield
                finally:
                    n = lib.axon_stop_nrt_profile(str(output_dir).encode())
                    print(f"ntff profile: {n} file(s) -> {output_dir}", file=sys.stderr)

            hook = _hook
    except OSError:
        pass
    mod = types.ModuleType("antenv.axon_hooks")
    mod.get_axon_ntff_profile_hook = lambda: hook
    mod.set_axon_ntff_profile_hook = lambda h: None
    sys.modules["antenv.axon_hooks"] = mod


_install_ntff_shim()

import concourse.bass as bass
import concourse.bacc as bacc
import concourse.mybir as mybir
import concourse.tile as tile
from concourse.bass_utils import run_bass_kernel_spmd

B, S, D = 8, 2048, 1024
P = 128           # partitions
NT = 512          # query (t) columns per block == one fp32 PSUM bank
TB = S // NT      # 4 t-blocks
HB = S // P       # 16 key (h) blocks
DC = D // P       # 8 feature chunks
TEMP = 1.0 / 32.0  # 1/sqrt(D)

_CACHE = {}


def _build_module():
    bf16 = mybir.dt.bfloat16
    f32 = mybir.dt.float32
    nc = bacc.Bacc("TRN2", target_bir_lowering=False, debug=False,
                   enable_asserts=False)

    headT = nc.dram_tensor("headT", [D, S], bf16, kind="ExternalInput")
    tailT = nc.dram_tensor("tailT", [D, S], bf16, kind="ExternalInput")
    tailN = nc.dram_tensor("tailN", [S, D], bf16, kind="ExternalInput")
    outT = nc.dram_tensor("outT", [D, S], f32, kind="ExternalOutput")

    headT_r = headT.rearrange("(dc p) h -> p dc h", p=P)
    tailT_r = tailT.rearrange("(dc p) t -> p dc t", p=P)
    tailN_r = tailN.rearrange("(hb p) d -> p hb d", p=P)

    with tile.TileContext(nc) as tc:
        with (
            tc.tile_pool(name="res", bufs=1) as res,
            tc.tile_pool(name="work", bufs=2) as work,
            tc.tile_pool(name="outp", bufs=3) as outp,
            tc.tile_pool(name="psS", bufs=2, space=bass.MemorySpace.PSUM) as psSp,
            tc.tile_pool(name="psO", bufs=2, space=bass.MemorySpace.PSUM) as psOp,
            tc.tile_pool(name="psD", bufs=2, space=bass.MemorySpace.PSUM) as psDp,
        ):
            headT_sb = res.tile([P, DC, S], bf16)
            tailT_sb = res.tile([P, DC, S], bf16)
            tailN_sb = res.tile([P, HB, D], bf16)
            ones_sb = res.tile([P, 1], bf16)
            nc.gpsimd.memset(ones_sb[:], 1.0)
            for dc in range(DC):
                nc.sync.dma_start(headT_sb[:, dc, :], headT_r[:, dc, :])
                nc.sync.dma_start(tailT_sb[:, dc, :], tailT_r[:, dc, :])
            for hb in range(HB):
                nc.sync.dma_start(tailN_sb[:, hb, :], tailN_r[:, hb, :])

            for tb in range(TB):
                tsl = slice(tb * NT, (tb + 1) * NT)
                E_t = work.tile([P, HB, NT], bf16, tag="E")

                # phase 1: S^T tiles (h on partitions) + exp -> E
                for hb in range(HB):
                    psS = psSp.tile([P, NT], f32, tag="psS")
                    for dc in range(DC):
                        nc.tensor.matmul(
                            psS[:],
                            headT_sb[:, dc, hb * P:(hb + 1) * P],
                            tailT_sb[:, dc, tsl],
                            start=(dc == 0),
                            stop=(dc == DC - 1),
                        )
                    nc.scalar.activation(
                        E_t[:, hb, :], psS[:],
                        mybir.ActivationFunctionType.Exp, scale=TEMP,
                    )

                # softmax denominator: ones^T @ E accumulated over h
                psD = psDp.tile([1, NT], f32, tag="psD")
                for hb in range(HB):
                    nc.tensor.matmul(
                        psD[:], ones_sb[:], E_t[:, hb, :],
                        start=(hb == 0), stop=(hb == HB - 1),
                    )
                rec = work.tile([1, NT], f32, tag="rec")
                nc.vector.reciprocal(rec[:], psD[:])
                rec_bc = work.tile([P, NT], f32, tag="recbc")
                nc.gpsimd.partition_broadcast(rec_bc[:], rec[:])

                # phase 2: O^T = V^T P^T (accumulate over h), normalize, store
                for dc in range(DC):
                    psO = psOp.tile([P, NT], f32, tag="psO")
                    for hb in range(HB):
                        nc.tensor.matmul(
                            psO[:],
                            tailN_sb[:, hb, dc * P:(dc + 1) * P],
                            E_t[:, hb, :],
                            start=(hb == 0), stop=(hb == HB - 1),
                        )
                    o_sb = outp.tile([P, NT], f32, tag="osb")
                    nc.vector.tensor_mul(o_sb[:], psO[:], rec_bc[:])
                    nc.sync.dma_start(outT[dc * P:(dc + 1) * P, tsl], o_sb[:])

    nc.compile()
    return nc


def kernel(head: np.ndarray, tail: np.ndarray) -> np.ndarray:
    assert head.shape == (B, S, D) and tail.shape == (B, S, D)
    if "nc" not in _CACHE:
        _CACHE["nc"] = _build_module()
    nc = _CACHE["nc"]

    head_bf = np.ascontiguousarray(head.astype(ml_dtypes.bfloat16))
    tail_bf = np.ascontiguousarray(tail.astype(ml_dtypes.bfloat16))
    in_maps = []
    for b in range(B):
        in_maps.append({
            "headT": np.ascontiguousarray(head_bf[b].T),
            "tailT": np.ascontiguousarray(tail_bf[b].T),
            "tailN": tail_bf[b],
        })

    trace = os.environ.get("BASS_ATTN_TRACE", "0") == "1"
    res = run_bass_kernel_spmd(nc, in_maps, core_ids=list(range(B)), trace=trace)
    _CACHE["last_result"] = res

    out = np.empty((B, S, D), dtype=np.float32)
    for b in range(B):
        out[b] = res.results[b]["outT"].T
    return out
